# revision 2
# baseline (speedup 1.0000x reference)
"""Modulated deformable conv (DCNv2) + eval-BN + ReLU on 8 TRN2 NeuronCores.

Sharding: 8 cores = (batch b in 0..3) x (image half h0 in {0, 48}).
Each core computes out[b, :, h0:h0+48, :] independently (no collectives).

Per-core pipeline (positions packed to 48x96 = 36 tiles of 128):
  - offset conv (3x3, 27ch) as accumulating matmuls over a padded c-major
    slab, emitted in 6 interleaved units with PE pos-transposes + field math
  - field math (DVE) -> bilinear tap weights wq (mask folded in) and
    quad-table row indices idxu
  - gather: per (tile, tap) one indirect SWDGE DMA fetches a 2KB "quad row"
    (4 corners x 256ch, bf16) per partition from a host-built table
  - tap combine: 4 per-partition-scalar products (23 DVE tensor_scalar in
    4x mode + 13 ACT activation-scale) + 3 wide DVE adds, written in
    (j, tile, c) column order into a group-wide val2
  - one batched dma_start_transpose per 4-tile group -> valt (ck, pos)
  - main conv: 36 accumulating matmuls per group (N=512); BN scale is
    folded into the conv weights host-side; bias+ReLU on ACT; bf16 out.
"""

import numpy as np
import ml_dtypes

import concourse.bass as bass
import concourse.tile as tile
import concourse.mybir as mybir
from concourse.bass_utils import run_bass_kernel_spmd

bf16 = mybir.dt.bfloat16
f32 = mybir.dt.float32
i16 = mybir.dt.int16

K = 9
PAD = 6
H = 96
HP = H + 2 * PAD  # 108
NPIX = HP * HP  # 11664
NT = 36  # pos tiles of 128 over packed 48x96
NPOS = NT * 128  # 4608
NK = NT * K  # 324
SLAB_ROWS = 53
SLAB = SLAB_ROWS * HP
NCHUNK = 12  # offset-conv chunks of 4 rows (384 pos)
CHUNK = 384
BN_EPS = 1e-5

_AF = mybir.ActivationFunctionType
_ALU = mybir.AluOpType

CLOSE_SETUP = True
TEST_PLAIN_DMA = False
TR_ENGINE = None
OUT_ENGINE = None
GP_BUFS = 3
VAL_BUFS = 2


def _build_program():
    nc = bass.Bass()
    xq_e = nc.dram_tensor("xq", [NPIX, 1024], bf16, kind="ExternalInput")
    xcm_e = nc.dram_tensor("xcm", [2, 128, SLAB], bf16, kind="ExternalInput")
    wofft_e = nc.dram_tensor("wofft", [128, 9 * 2 * 27], bf16, kind="ExternalInput")
    w2_e = nc.dram_tensor("w2", [128, 18 * 2 * 128], bf16, kind="ExternalInput")
    ident_e = nc.dram_tensor("ident", [128, 128], f32, kind="ExternalInput")
    basey_e = nc.dram_tensor("basey", [128, NK], f32, kind="ExternalInput")
    basex_e = nc.dram_tensor("basex", [128, NK], f32, kind="ExternalInput")
    basem_e = nc.dram_tensor("basem", [128, NK], f32, kind="ExternalInput")
    bnw_e = nc.dram_tensor("bnw", [128, 2], f32, kind="ExternalInput")
    bnb_e = nc.dram_tensor("bnb", [128, 2], f32, kind="ExternalInput")
    out_e = nc.dram_tensor("out", [256, NPOS], bf16, kind="ExternalOutput")

    with tile.TileContext(nc) as tc:
        with (
            tc.tile_pool(name="const", bufs=1) as cp,
        ):
            setupctx = tc.tile_pool(name="setup", bufs=1)
            fp = setupctx.__enter__()
            # ---- load constants ----
            xcm = [fp.tile([128, SLAB], bf16, name=f"xcm{c}", tag=f"xcm{c}") for c in range(2)]
            for c in range(2):
                nc.sync.dma_start(xcm[c][:], xcm_e[c])
            wofft = fp.tile([128, 9 * 2 * 27], bf16)
            nc.sync.dma_start(wofft[:], wofft_e[:])
            w2 = cp.tile([128, 18 * 2 * 128], bf16)
            nc.sync.dma_start(w2[:], w2_e[:])
            ident = fp.tile([128, 128], f32)
            nc.sync.dma_start(ident[:], ident_e[:])
            basey = fp.tile([128, NK], f32)
            nc.sync.dma_start(basey[:], basey_e[:])
            basex = fp.tile([128, NK], f32)
            nc.sync.dma_start(basex[:], basex_e[:])
            basem = fp.tile([128, NK], f32)
            nc.sync.dma_start(basem[:], basem_e[:])
            bnw = cp.tile([128, 2], f32)
            nc.sync.dma_start(bnw[:], bnw_e[:])
            bnb = cp.tile([128, 2], f32)
            nc.sync.dma_start(bnb[:], bnb_e[:])

            # ---- setup in two halves so gathers start after half the conv ----
            convtr = tc.tile_pool(name="conv_ps", bufs=2, space="PSUM")
            convp = convtr.__enter__()
            trctx = tc.tile_pool(name="tr_ps", bufs=2, space="PSUM")
            trp = trctx.__enter__()
            off_cm = fp.tile([32, NPOS], f32)
            offpk = fp.tile([128, NT * 32], f32)
            pyt = fp.tile([128, NK], f32)
            pxt = fp.tile([128, NK], f32)
            fy = fp.tile([128, NK], f32)
            fx = fp.tile([128, NK], f32)
            y0 = fp.tile([128, NK], f32)
            x0 = fp.tile([128, NK], f32)
            msk = fp.tile([128, NK], f32)
            bb = fp.tile([128, NK], f32)
            aa = fp.tile([128, NK], f32)
            wx0 = fp.tile([128, NK], f32)
            idxf = fp.tile([128, NK], f32)
            idxu = cp.tile([128, NK], mybir.dt.uint32)
            wq = cp.tile([128, NK * 4], f32)
            yi = fp.tile([128, NK], mybir.dt.int32)
            xi = fp.tile([128, NK], mybir.dt.int32)
            gt = fp.tile([128, NK], f32)
            taps = [(dy, dx) for dy in (-1, 0, 1) for dx in (-1, 0, 1)]

            o3 = offpk[:].rearrange("p (t c) -> p t c", c=32)

            def setup_half(h):
                # offset conv for chunks [2h, 2h+2) -> off_cm cols
                for ci in range(2 * h, 2 * h + 2):
                    r0 = ci * 4
                    ps = convp.tile([32, CHUNK], f32, tag="convps")
                    n = 0
                    for ti, (dy, dx) in enumerate(taps):
                        for ch in range(2):
                            base = (2 + dy + r0) * HP + 6 + dx
                            rhs = xcm[ch][:, base:base + 4 * HP].rearrange(
                                "p (r w) -> p r w", w=HP)[:, :, :96]
                            nc.tensor.matmul(
                                ps[:27, :].rearrange("p (r w) -> p r w", w=96),
                                wofft[:, (ti * 2 + ch) * 27:(ti * 2 + ch) * 27 + 27],
                                rhs,
                                start=(n == 0),
                                stop=(n == 17),
                            )
                            n += 1
                    nc.vector.tensor_copy(
                        off_cm[:27, ci * CHUNK:(ci + 1) * CHUNK], ps[:27, :])
                # transpose to pos-major for tiles [6h, 6h+6)
                for t in range(6 * h, 6 * h + 6):
                    pst = trp.tile([128, 32], f32, tag="trps")
                    nc.tensor.transpose(
                        pst[:, :32], off_cm[:32, t * 128:(t + 1) * 128],
                        ident[:32, :32])
                    nc.vector.tensor_copy(offpk[:, t * 32:(t + 1) * 32], pst[:])
                # field math on slot range [54h, 54h+54)
                s = slice(54 * h, 54 * h + 54)
                ts = slice(6 * h, 6 * h + 6)
                dy_all = o3[:, ts, 0:18:2]
                dx_all = o3[:, ts, 1:18:2]
                ml_all = o3[:, ts, 18:27]

                def v3(t128):
                    return t128[:, s].rearrange("p (t k) -> p t k", k=K)

                bs = basey[:, s].rearrange("p (t k) -> p t k", k=K)
                nc.vector.tensor_add(v3(pyt), dy_all, bs)
                bs = basex[:, s].rearrange("p (t k) -> p t k", k=K)
                nc.vector.tensor_add(v3(pxt), dx_all, bs)
                nc.vector.tensor_copy(yi[:, s], pyt[:, s])
                nc.vector.tensor_copy(y0[:, s], yi[:, s])
                nc.vector.tensor_tensor(gt[:, s], y0[:, s], pyt[:, s], op=_ALU.is_gt)
                nc.vector.tensor_sub(y0[:, s], y0[:, s], gt[:, s])
                nc.vector.tensor_sub(fy[:, s], pyt[:, s], y0[:, s])
                nc.vector.tensor_copy(xi[:, s], pxt[:, s])
                nc.vector.tensor_copy(x0[:, s], xi[:, s])
                nc.vector.tensor_tensor(gt[:, s], x0[:, s], pxt[:, s], op=_ALU.is_gt)
                nc.vector.tensor_sub(x0[:, s], x0[:, s], gt[:, s])
                nc.vector.tensor_sub(fx[:, s], pxt[:, s], x0[:, s])
                nc.vector.tensor_scalar(y0[:, s], y0[:, s], 0.0, float(HP - 2), op0=_ALU.max, op1=_ALU.min)
                nc.vector.tensor_scalar(x0[:, s], x0[:, s], 0.0, float(HP - 2), op0=_ALU.max, op1=_ALU.min)
                nc.vector.tensor_scalar(idxf[:, s], y0[:, s], float(HP), None, op0=_ALU.mult)
                nc.vector.tensor_add(idxf[:, s], idxf[:, s], x0[:, s])
                nc.vector.tensor_copy(idxu[:, s], idxf[:, s])
                bs = basem[:, s].rearrange("p (t k) -> p t k", k=K)
                nc.vector.tensor_add(v3(msk), ml_all, bs)
                nc.scalar.activation(msk[:, s], msk[:, s], _AF.Sigmoid)
                nc.vector.tensor_mul(bb[:, s], msk[:, s], fy[:, s])
                nc.vector.tensor_sub(aa[:, s], msk[:, s], bb[:, s])
                nc.vector.tensor_scalar(wx0[:, s], fx[:, s], -1.0, 1.0, op0=_ALU.mult, op1=_ALU.add)
                w3h = wq[:, 4 * 54 * h:4 * 54 * (h + 1)].rearrange(
                    "p (n j) -> p n j", j=4)
                nc.vector.tensor_mul(w3h[:, :, 0], aa[:, s], wx0[:, s])
                nc.vector.tensor_mul(w3h[:, :, 1], bb[:, s], wx0[:, s])
                nc.vector.tensor_mul(w3h[:, :, 2], aa[:, s], fx[:, s])
                nc.vector.tensor_mul(w3h[:, :, 3], bb[:, s], fx[:, s])

            for _h in range(6):
                setup_half(_h)
            trctx.__exit__(None, None, None)
            convtr.__exit__(None, None, None)
            if CLOSE_SETUP:
                setupctx.__exit__(None, None, None)

            gp_ctx = tc.tile_pool(name="gpool", bufs=GP_BUFS)
            gp = gp_ctx.__enter__()
            vp_ctx = tc.tile_pool(name="val", bufs=VAL_BUFS)
            vp = vp_ctx.__enter__()
            pp_ctx = tc.tile_pool(name="prod", bufs=VAL_BUFS)
            pp = pp_ctx.__enter__()
            vtp_ctx = tc.tile_pool(name="valt", bufs=2)
            vtp = vtp_ctx.__enter__()
            outp_ctx = tc.tile_pool(name="out_ps", bufs=2, space="PSUM")
            outp = outp_ctx.__enter__()
            osb_ctx = tc.tile_pool(name="osb", bufs=4)
            osb_p = osb_ctx.__enter__()

            # ---- main loop: software-pipelined with gathers LAG tiles ahead ----
            LAG = GP_BUFS - 1
            gbufs = {}
            valts = {}

            def emit_gather(t):
                g_t = gp.tile([128, 9 * 1024], bf16, tag="g")
                gbufs[t] = g_t
                for kk in range(K):
                    slot = t * K + kk
                    nc.gpsimd.indirect_dma_start(
                        out=g_t[:, kk * 1024:(kk + 1) * 1024],
                        out_offset=None,
                        in_=xq_e[:],
                        in_offset=bass.IndirectOffsetOnAxis(
                            ap=idxu[:, slot:slot + 1], axis=0
                        ),
                    )

            val2s = {}

            def emit_combine(t):
                g_t = gbufs.pop(t)
                gg = t // 4
                u = t % 4
                if u == 0:
                    val2s[gg] = vp.tile([128, 9216], bf16, name="val2", tag="val2")
                val2 = val2s[gg]
                # val2 columns ordered (j:18, tile:4, c:128) so one batched
                # dma_start_transpose per group lands valt = (jt-major, pos).
                p1 = pp.tile([128, 2304], bf16, tag="p1")
                p2 = pp.tile([128, 2304], bf16, tag="p2")
                p3 = pp.tile([128, 2304], bf16, tag="p3")
                for kk in range(K):
                    slot = t * K + kk
                    q = g_t[:, kk * 1024:(kk + 1) * 1024]
                    c = kk * 256
                    vdst = val2[:, 2 * kk * 512 + u * 128:].rearrange(
                        "p (a b) -> p a b", b=128)[:, 0:5:4, :]
                    nc.vector.tensor_scalar(
                        vdst, q[:, 0:256].rearrange("p (a b) -> p a b", b=128),
                        wq[:, slot * 4:slot * 4 + 1], None, op0=_ALU.mult)
                    if kk < 4:
                        nc.scalar.activation(
                            p1[:, c:c + 256], q[:, 256:512], _AF.Copy,
                            scale=wq[:, slot * 4 + 1:slot * 4 + 2])
                    else:
                        nc.vector.tensor_scalar(
                            p1[:, c:c + 256], q[:, 256:512],
                            wq[:, slot * 4 + 1:slot * 4 + 2], None, op0=_ALU.mult)
                    nc.scalar.activation(
                        p2[:, c:c + 256], q[:, 512:768], _AF.Copy,
                        scale=wq[:, slot * 4 + 2:slot * 4 + 3])
                    nc.vector.tensor_scalar(
                        p3[:, c:c + 256], q[:, 768:1024],
                        wq[:, slot * 4 + 3:slot * 4 + 4], None, op0=_ALU.mult)
                # adds: three wide TT passes into the strided val2 tile-slice
                vsl = val2[:, u * 128:].rearrange(
                    "p (a b) -> p a b", b=128)[:, 0:69:4, :]
                pv1 = p1[:].rearrange("p (a b) -> p a b", b=128)
                pv2 = p2[:].rearrange("p (a b) -> p a b", b=128)
                pv3 = p3[:].rearrange("p (a b) -> p a b", b=128)
                nc.vector.tensor_add(vsl, vsl, pv1)
                nc.vector.tensor_add(pv2, pv2, pv3)
                nc.vector.tensor_add(vsl, vsl, pv2)
                if u == 3:
                    valts[gg] = vtp.tile(
                        [128, 18 * 512], bf16, name="valt", tag="valt")
                    vo = valts[gg][:].rearrange("p (j t c) -> p j t c", t=4, c=128)
                    nc.sync.dma_start_transpose(vo, val2s.pop(gg)[:])

            def emit_matmuls(g):
                tlo = g * 4
                valt = valts.pop(g)
                pso = [outp.tile([128, 512], f32, name=f"pso{oh}", tag=f"ops{oh}") for oh in range(2)]
                for oh in range(2):
                    for j in range(18):
                        nc.tensor.matmul(
                            pso[oh][:],
                            w2[:, (j * 2 + oh) * 128:(j * 2 + oh) * 128 + 128],
                            valt[:, j * 512:(j + 1) * 512],
                            start=(j == 0),
                            stop=(j == 17),
                        )
                    ob = osb_p.tile([128, 512], bf16, tag="ob")
                    nc.scalar.activation(
                        ob[:], pso[oh][:], _AF.Relu,
                        bias=bnb[:, oh:oh + 1],
                    )
                    nc.sync.dma_start(
                        out_e[oh * 128:(oh + 1) * 128, tlo * 128:tlo * 128 + 512],
                        ob[:],
                    )

            for t in range(NT + LAG):
                if t < NT:
                    emit_gather(t)
                if t >= LAG:
                    emit_combine(t - LAG)
                    if (t - LAG) % 4 == 3:
                        emit_matmuls((t - LAG) // 4)
            osb_ctx.__exit__(None, None, None)
            outp_ctx.__exit__(None, None, None)
            vtp_ctx.__exit__(None, None, None)
            pp_ctx.__exit__(None, None, None)
            vp_ctx.__exit__(None, None, None)
            gp_ctx.__exit__(None, None, None)
    _split_multi_waits(nc)
    return nc


def _split_multi_waits(nc, maxw=1):
    """The walrus build here rejects instructions with >1 semaphore wait;
    hoist excess waits onto standalone event-semaphore instructions."""
    n_fixed = 0
    for fn in nc.m.functions:
        for blk in fn.blocks:
            il = blk.instructions
            i = 0
            while i < len(il):
                inst = il[i]
                si = inst.sync_info
                if si is not None and len(si.on_wait) > maxw:
                    waits = list(si.on_wait)
                    keep = waits[:maxw - 1] if maxw > 1 else []
                    hoist = waits[len(keep):-1] if maxw > 1 else waits[:-1]
                    inst.sync_info = mybir.SyncInfo(
                        on_wait=keep + [waits[-1]], on_update=list(si.on_update)
                    )
                    for j, w in enumerate(hoist):
                        ev = mybir.InstEventSemaphore(
                            name=f"{inst.name}-hw{j}", ins=[], outs=[]
                        )
                        ev.engine = inst.engine
                        ev.sync_info = mybir.SyncInfo(on_wait=[w], on_update=[])
                        il.insert(i, ev)
                        i += 1
                    n_fixed += 1
                i += 1
    return n_fixed


# ---------------- host side ----------------

def _prep_inputs(input_x, w_off, b_off, w, b, gamma, beta, rmean, rvar):
    B = input_x.shape[0]
    x = np.asarray(input_x, np.float32)
    xbf = x.astype(ml_dtypes.bfloat16)
    xp = np.zeros((B, 256, HP, HP), ml_dtypes.bfloat16)
    xp[:, :, PAD:PAD + H, PAD:PAD + H] = xbf
    xpp = np.zeros((B, 256, HP + 1, HP + 1), ml_dtypes.bfloat16)
    xpp[:, :, :HP, :HP] = xp
    ys, xs = np.divmod(np.arange(NPIX), HP)
    xq = np.empty((B, NPIX, 4, 256), ml_dtypes.bfloat16)
    for j, (dy, dx) in enumerate(((0, 0), (1, 0), (0, 1), (1, 1))):
        xq[:, :, j, :] = xpp[:, :, ys + dy, xs + dx].transpose(0, 2, 1)
    xq = xq.reshape(B, NPIX, 1024)

    wofft = np.empty((128, 9, 2, 27), ml_dtypes.bfloat16)
    wf = np.asarray(w_off, np.float32)
    for ti in range(9):
        ty, tx = divmod(ti, 3)
        for ch in range(2):
            wofft[:, ti, ch, :] = wf[:, ch * 128:(ch + 1) * 128, ty, tx].T.astype(
                ml_dtypes.bfloat16)
    wofft = wofft.reshape(128, 9 * 2 * 27)

    scale_o = (np.asarray(gamma, np.float32)
               / np.sqrt(np.asarray(rvar, np.float32) + BN_EPS))
    wr = np.asarray(w, np.float32).reshape(256, 256, 9)
    wr = wr * scale_o[:, None, None]  # fold BN scale into conv weights
    w2 = np.empty((128, 18, 2, 128), ml_dtypes.bfloat16)
    for kk in range(9):
        for ch in range(2):
            j = 2 * kk + ch
            for oh in range(2):
                w2[:, j, oh, :] = wr[oh * 128:(oh + 1) * 128,
                                     ch * 128:(ch + 1) * 128, kk].T.astype(
                    ml_dtypes.bfloat16)
    w2 = w2.reshape(128, 18 * 2 * 128)

    ident = np.eye(128, dtype=np.float32)

    scale = (np.asarray(gamma, np.float32)
             / np.sqrt(np.asarray(rvar, np.float32) + BN_EPS))
    bias_tot = (np.asarray(b, np.float32) * scale
                + np.asarray(beta, np.float32)
                - np.asarray(rmean, np.float32) * scale)
    bnw = scale.reshape(2, 128).T.copy()  # unused on device now
    bnb = bias_tot.reshape(2, 128).T.copy()

    ky = (np.arange(K) // 3 - 1).astype(np.float32)
    kx = (np.arange(K) % 3 - 1).astype(np.float32)
    boff = np.asarray(b_off, np.float32)

    per_core = []
    for core in range(8):
        bidx_core, half = divmod(core, 2)
        h0 = half * 48
        # packed positions: p -> (y, x) = (p//96, p%96); padded coords
        # (h0+PAD+y, PAD+x)
        p = np.arange(128)[:, None] + 128 * np.arange(NT)[None, :]  # (128, NT)
        ypad = h0 + PAD + p // 96
        xpad = PAD + p % 96
        basey = (ypad[:, :, None] + ky[None, None, :]
                 + boff[0:18:2][None, None, :]).astype(np.float32)
        basex = (xpad[:, :, None] + kx[None, None, :]
                 + boff[1:18:2][None, None, :]).astype(np.float32)
        basem = np.broadcast_to(boff[18:27][None, None, :], basey.shape).astype(np.float32)
        slab = np.ascontiguousarray(
            xp[bidx_core, :, h0 + 4:h0 + 4 + SLAB_ROWS, :].reshape(256, SLAB)
            .reshape(2, 128, SLAB))
        per_core.append({
            "xq": np.ascontiguousarray(xq[bidx_core]),
            "xcm": slab,
            "wofft": wofft,
            "w2": w2,
            "ident": ident,
            "basey": np.ascontiguousarray(basey.reshape(128, NK)),
            "basex": np.ascontiguousarray(basex.reshape(128, NK)),
            "basem": np.ascontiguousarray(basem.reshape(128, NK)),
            "bnw": np.ascontiguousarray(bnw),
            "bnb": np.ascontiguousarray(bnb),
        })
    return per_core


_PROG_CACHE = {}


def _get_program():
    if "nc" not in _PROG_CACHE:
        _PROG_CACHE["nc"] = _build_program()
    return _PROG_CACHE["nc"]


def kernel(**inputs):
    return _run(inputs, trace=False)[0]


def _run(inputs, trace=False):
    per_core = _prep_inputs(**inputs)
    nc = _get_program()
    res = run_bass_kernel_spmd(nc, per_core, list(range(8)), trace=trace)
    out = np.empty((4, 256, 96, 96), np.float32)
    for core in range(8):
        bidx_core, half = divmod(core, 2)
        h0 = half * 48
        out[bidx_core, :, h0:h0 + 48, :] = (
            res.results[core]["out"].astype(np.float32).reshape(256, 48, 96))
    return out, res.exec_time_ns


# revision 3
# speedup vs baseline: 1.2385x; 1.2385x over previous
"""Modulated deformable conv (DCNv2) + eval-BN + ReLU on 8 TRN2 NeuronCores.

Sharding: 8 cores = (batch b in 0..3) x (image half h0 in {0, 48}).
Each core computes out[b, :, h0:h0+48, :] independently (no collectives).

Per-core pipeline (positions packed to 48x96 = 36 tiles of 128):
  - offset conv (3x3, 27ch) emitted as 6 independent units (conv matmuls ->
    PE pos-transposes -> DVE field math -> tap weights wq + gather indices
    idxu); all tile pools coexist so unit 0's gathers overlap later units
  - gather: per (tile, tap) one indirect SWDGE DMA fetches a 2KB "quad row"
    (4 bilinear corners x 256ch, bf16) per partition from a host-built table
  - tap combine: 4 per-partition-scalar products (23 DVE tensor_scalar in
    4x mode + 13 ACT activation-scale) + 3 wide DVE adds, written in
    (j, tile, c) column order into a group-wide val2
  - one batched dma_start_transpose per 4-tile group -> valt (ck, pos)
  - main conv: 36 accumulating matmuls per group (N=512); BN scale folded
    into the conv weights host-side; bias+ReLU on ACT; bf16 out, one DMA
    per group.
"""

import numpy as np
import ml_dtypes

import concourse.bass as bass
import concourse.tile as tile
import concourse.mybir as mybir
from concourse.bass_utils import run_bass_kernel_spmd

bf16 = mybir.dt.bfloat16
f32 = mybir.dt.float32
i16 = mybir.dt.int16

K = 9
PAD = 6
H = 96
HP = H + 2 * PAD  # 108
NPIX = HP * HP  # 11664
NT = 36  # pos tiles of 128 over packed 48x96
NPOS = NT * 128  # 4608
NK = NT * K  # 324
SLAB_ROWS = 53
SLAB = SLAB_ROWS * HP
NCHUNK = 12  # offset-conv chunks of 4 rows (384 pos)
CHUNK = 384
BN_EPS = 1e-5

_AF = mybir.ActivationFunctionType
_ALU = mybir.AluOpType

CLOSE_SETUP = True
TEST_PLAIN_DMA = False
TR_ENGINE = None
OUT_ENGINE = None
GP_BUFS = 2
VAL_BUFS = 2


def _build_program():
    nc = bass.Bass()
    xq_e = nc.dram_tensor("xq", [NPIX, 1024], bf16, kind="ExternalInput")
    xcm_e = nc.dram_tensor("xcm", [2, 128, SLAB], bf16, kind="ExternalInput")
    wofft_e = nc.dram_tensor("wofft", [128, 9 * 2 * 27], bf16, kind="ExternalInput")
    w2_e = nc.dram_tensor("w2", [128, 18 * 2 * 128], bf16, kind="ExternalInput")
    ident_e = nc.dram_tensor("ident", [128, 128], f32, kind="ExternalInput")
    basey_e = nc.dram_tensor("basey", [128, NK], f32, kind="ExternalInput")
    basex_e = nc.dram_tensor("basex", [128, NK], f32, kind="ExternalInput")
    basem_e = nc.dram_tensor("basem", [128, NK], f32, kind="ExternalInput")
    bnw_e = nc.dram_tensor("bnw", [128, 2], f32, kind="ExternalInput")
    bnb_e = nc.dram_tensor("bnb", [128, 2], f32, kind="ExternalInput")
    out_e = nc.dram_tensor("out", [256, NPOS], bf16, kind="ExternalOutput")

    with tile.TileContext(nc) as tc:
        with (
            tc.tile_pool(name="const", bufs=1) as cp,
        ):
            setupctx = tc.tile_pool(name="setup", bufs=1)
            fp = setupctx.__enter__()
            # ---- load constants ----
            xcm = [fp.tile([128, SLAB], bf16, name=f"xcm{c}", tag=f"xcm{c}") for c in range(2)]
            _xsp = [0, 19 * HP, 37 * HP, SLAB]
            for c in range(2):
                for _a, _b in zip(_xsp[:-1], _xsp[1:]):
                    nc.sync.dma_start(xcm[c][:, _a:_b], xcm_e[c, :, _a:_b])
            wofft = fp.tile([128, 9 * 2 * 27], bf16)
            nc.sync.dma_start(wofft[:], wofft_e[:])
            w2 = cp.tile([128, 18 * 2 * 128], bf16)
            nc.sync.dma_start(w2[:], w2_e[:])
            ident = fp.tile([128, 128], f32)
            nc.sync.dma_start(ident[:], ident_e[:])
            basey = fp.tile([128, NK], f32)
            nc.sync.dma_start(basey[:], basey_e[:])
            basex = fp.tile([128, NK], f32)
            nc.sync.dma_start(basex[:], basex_e[:])
            basem = fp.tile([128, NK], f32)
            nc.sync.dma_start(basem[:], basem_e[:])
            bnw = cp.tile([128, 2], f32)
            nc.sync.dma_start(bnw[:], bnw_e[:])
            bnb = cp.tile([128, 2], f32)
            nc.sync.dma_start(bnb[:], bnb_e[:])

            # ---- setup: 6 units of 6 tiles, small rotating per-unit tiles so
            # the setup pool coexists with the main-loop pools (no closure
            # barrier -- gathers start right after unit 0's field math). ----
            convtr = tc.tile_pool(name="conv_ps", bufs=2, space="PSUM")
            convp = convtr.__enter__()
            trctx = tc.tile_pool(name="tr_ps", bufs=2, space="PSUM")
            trp = trctx.__enter__()
            fup_ctx = tc.tile_pool(name="funit", bufs=2)
            fup = fup_ctx.__enter__()
            idxu = cp.tile([128, NK], mybir.dt.uint32)
            wq = cp.tile([128, NK * 4], f32)
            taps = [(dy, dx) for dy in (-1, 0, 1) for dx in (-1, 0, 1)]
            UT = 6            # tiles per unit
            UP = UT * 128     # positions per unit
            US = UT * K       # slots per unit

            def setup_half(h):
                off_u = fup.tile([32, UP], f32, name="off_u", tag="off_u")
                # offset conv for chunks [2h, 2h+2) -> off_u cols
                for ci in range(2):
                    r0 = (2 * h + ci) * 4
                    ps = convp.tile([32, CHUNK], f32, tag="convps")
                    n = 0
                    for ti, (dy, dx) in enumerate(taps):
                        for ch in range(2):
                            base = (2 + dy + r0) * HP + 6 + dx
                            rhs = xcm[ch][:, base:base + 4 * HP].rearrange(
                                "p (r w) -> p r w", w=HP)[:, :, :96]
                            nc.tensor.matmul(
                                ps[:27, :].rearrange("p (r w) -> p r w", w=96),
                                wofft[:, (ti * 2 + ch) * 27:(ti * 2 + ch) * 27 + 27],
                                rhs,
                                start=(n == 0),
                                stop=(n == 17),
                            )
                            n += 1
                    nc.vector.tensor_copy(
                        off_u[:27, ci * CHUNK:(ci + 1) * CHUNK], ps[:27, :])
                # transpose to pos-major for this unit's 6 tiles
                offpk = fup.tile([128, UT * 32], f32, name="offpk", tag="offpk")
                for tt in range(UT):
                    pst = trp.tile([128, 32], f32, tag="trps")
                    nc.tensor.transpose(
                        pst[:, :32], off_u[:32, tt * 128:(tt + 1) * 128],
                        ident[:32, :32])
                    nc.vector.tensor_copy(offpk[:, tt * 32:(tt + 1) * 32], pst[:])
                # field math on this unit's 54 slots
                o3 = offpk[:].rearrange("p (t c) -> p t c", c=32)
                dy_all = o3[:, :, 0:18:2]
                dx_all = o3[:, :, 1:18:2]
                ml_all = o3[:, :, 18:27]
                s = slice(US * h, US * h + US)
                pyt = fup.tile([128, US], f32, name="pyt", tag="pyt")
                pxt = fup.tile([128, US], f32, name="pxt", tag="pxt")
                fy = fup.tile([128, US], f32, name="fy", tag="fy")
                fx = fup.tile([128, US], f32, name="fx", tag="fx")
                y0 = fup.tile([128, US], f32, name="y0", tag="y0")
                x0 = fup.tile([128, US], f32, name="x0", tag="x0")
                msk = fup.tile([128, US], f32, name="msk", tag="msk")
                bb = fup.tile([128, US], f32, name="bb", tag="bb")
                aa = fup.tile([128, US], f32, name="aa", tag="aa")
                wx0 = fup.tile([128, US], f32, name="wx0", tag="wx0")
                idxf = fup.tile([128, US], f32, name="idxf", tag="idxf")
                yi = fup.tile([128, US], mybir.dt.int32, name="yi", tag="yi")
                xi = fup.tile([128, US], mybir.dt.int32, name="xi", tag="xi")
                gt = fup.tile([128, US], f32, name="gt", tag="gt")

                def v3(t128):
                    return t128[:].rearrange("p (t k) -> p t k", k=K)

                bs = basey[:, s].rearrange("p (t k) -> p t k", k=K)
                nc.vector.tensor_add(v3(pyt), dy_all, bs)
                bs = basex[:, s].rearrange("p (t k) -> p t k", k=K)
                nc.vector.tensor_add(v3(pxt), dx_all, bs)
                nc.vector.tensor_copy(yi[:], pyt[:])
                nc.vector.tensor_copy(y0[:], yi[:])
                nc.vector.tensor_tensor(gt[:], y0[:], pyt[:], op=_ALU.is_gt)
                nc.vector.tensor_sub(y0[:], y0[:], gt[:])
                nc.vector.tensor_sub(fy[:], pyt[:], y0[:])
                nc.vector.tensor_copy(xi[:], pxt[:])
                nc.vector.tensor_copy(x0[:], xi[:])
                nc.vector.tensor_tensor(gt[:], x0[:], pxt[:], op=_ALU.is_gt)
                nc.vector.tensor_sub(x0[:], x0[:], gt[:])
                nc.vector.tensor_sub(fx[:], pxt[:], x0[:])
                nc.vector.tensor_scalar(y0[:], y0[:], 0.0, float(HP - 2), op0=_ALU.max, op1=_ALU.min)
                nc.vector.tensor_scalar(x0[:], x0[:], 0.0, float(HP - 2), op0=_ALU.max, op1=_ALU.min)
                nc.vector.tensor_scalar(idxf[:], y0[:], float(HP), None, op0=_ALU.mult)
                nc.vector.tensor_add(idxf[:], idxf[:], x0[:])
                nc.vector.tensor_copy(idxu[:, s], idxf[:])
                bs = basem[:, s].rearrange("p (t k) -> p t k", k=K)
                nc.vector.tensor_add(v3(msk), ml_all, bs)
                nc.scalar.activation(msk[:], msk[:], _AF.Sigmoid)
                nc.vector.tensor_mul(bb[:], msk[:], fy[:])
                nc.vector.tensor_sub(aa[:], msk[:], bb[:])
                nc.vector.tensor_scalar(wx0[:], fx[:], -1.0, 1.0, op0=_ALU.mult, op1=_ALU.add)
                w3h = wq[:, 4 * US * h:4 * US * (h + 1)].rearrange(
                    "p (n j) -> p n j", j=4)
                nc.vector.tensor_mul(w3h[:, :, 0], aa[:], wx0[:])
                nc.vector.tensor_mul(w3h[:, :, 1], bb[:], wx0[:])
                nc.vector.tensor_mul(w3h[:, :, 2], aa[:], fx[:])
                nc.vector.tensor_mul(w3h[:, :, 3], bb[:], fx[:])

            gp_ctx = tc.tile_pool(name="gpool", bufs=GP_BUFS)
            gp = gp_ctx.__enter__()
            vp_ctx = tc.tile_pool(name="val", bufs=VAL_BUFS)
            vp = vp_ctx.__enter__()
            pp_ctx = tc.tile_pool(name="prod", bufs=VAL_BUFS)
            pp = pp_ctx.__enter__()
            vtp_ctx = tc.tile_pool(name="valt", bufs=2)
            vtp = vtp_ctx.__enter__()
            outp_ctx = tc.tile_pool(name="out_ps", bufs=2, space="PSUM")
            outp = outp_ctx.__enter__()
            osb_ctx = tc.tile_pool(name="osb", bufs=4)
            osb_p = osb_ctx.__enter__()
            for _h in range(6):
                setup_half(_h)


            # ---- main loop: software-pipelined with gathers LAG tiles ahead ----
            LAG = GP_BUFS - 1
            gbufs = {}
            valts = {}

            def emit_gather(t):
                g_t = gp.tile([128, 9 * 1024], bf16, tag="g")
                gbufs[t] = g_t
                for kk in range(K):
                    slot = t * K + kk
                    nc.gpsimd.indirect_dma_start(
                        out=g_t[:, kk * 1024:(kk + 1) * 1024],
                        out_offset=None,
                        in_=xq_e[:],
                        in_offset=bass.IndirectOffsetOnAxis(
                            ap=idxu[:, slot:slot + 1], axis=0
                        ),
                    )

            val2s = {}

            def emit_combine(t):
                g_t = gbufs.pop(t)
                gg = t // 4
                u = t % 4
                if u == 0:
                    val2s[gg] = vp.tile([128, 9216], bf16, name="val2", tag="val2")
                val2 = val2s[gg]
                # val2 columns ordered (j:18, tile:4, c:128) so one batched
                # dma_start_transpose per group lands valt = (jt-major, pos).
                p1 = pp.tile([128, 2304], bf16, tag="p1")
                p2 = pp.tile([128, 2304], bf16, tag="p2")
                p3 = pp.tile([128, 2304], bf16, tag="p3")
                for kk in range(K):
                    slot = t * K + kk
                    q = g_t[:, kk * 1024:(kk + 1) * 1024]
                    c = kk * 256
                    vdst = val2[:, 2 * kk * 512 + u * 128:].rearrange(
                        "p (a b) -> p a b", b=128)[:, 0:5:4, :]
                    nc.vector.tensor_scalar(
                        vdst, q[:, 0:256].rearrange("p (a b) -> p a b", b=128),
                        wq[:, slot * 4:slot * 4 + 1], None, op0=_ALU.mult)
                    if kk < 4:
                        nc.scalar.activation(
                            p1[:, c:c + 256], q[:, 256:512], _AF.Copy,
                            scale=wq[:, slot * 4 + 1:slot * 4 + 2])
                    else:
                        nc.vector.tensor_scalar(
                            p1[:, c:c + 256], q[:, 256:512],
                            wq[:, slot * 4 + 1:slot * 4 + 2], None, op0=_ALU.mult)
                    nc.scalar.activation(
                        p2[:, c:c + 256], q[:, 512:768], _AF.Copy,
                        scale=wq[:, slot * 4 + 2:slot * 4 + 3])
                    nc.vector.tensor_scalar(
                        p3[:, c:c + 256], q[:, 768:1024],
                        wq[:, slot * 4 + 3:slot * 4 + 4], None, op0=_ALU.mult)
                # adds: three wide TT passes into the strided val2 tile-slice
                vsl = val2[:, u * 128:].rearrange(
                    "p (a b) -> p a b", b=128)[:, 0:69:4, :]
                pv1 = p1[:].rearrange("p (a b) -> p a b", b=128)
                pv2 = p2[:].rearrange("p (a b) -> p a b", b=128)
                pv3 = p3[:].rearrange("p (a b) -> p a b", b=128)
                nc.vector.tensor_add(vsl, vsl, pv1)
                nc.vector.tensor_add(pv2, pv2, pv3)
                nc.vector.tensor_add(vsl, vsl, pv2)
                if u == 3:
                    valts[gg] = vtp.tile(
                        [128, 18 * 512], bf16, name="valt", tag="valt")
                    vo = valts[gg][:].rearrange("p (j t c) -> p j t c", t=4, c=128)
                    nc.sync.dma_start_transpose(vo, val2s.pop(gg)[:])

            def emit_matmuls(g):
                tlo = g * 4
                valt = valts.pop(g)
                pso = [outp.tile([128, 512], f32, name=f"pso{oh}", tag=f"ops{oh}") for oh in range(2)]
                ob = osb_p.tile([128, 1024], bf16, tag="ob")
                for oh in range(2):
                    for j in range(18):
                        nc.tensor.matmul(
                            pso[oh][:],
                            w2[:, (j * 2 + oh) * 128:(j * 2 + oh) * 128 + 128],
                            valt[:, j * 512:(j + 1) * 512],
                            start=(j == 0),
                            stop=(j == 17),
                        )
                    nc.scalar.activation(
                        ob[:, oh * 512:(oh + 1) * 512], pso[oh][:], _AF.Relu,
                        bias=bnb[:, oh:oh + 1],
                    )
                nc.sync.dma_start(
                    out_e[:, tlo * 128:tlo * 128 + 512].rearrange(
                        "(oh p) n -> p oh n", oh=2),
                    ob[:].rearrange("p (oh n) -> p oh n", n=512),
                )

            for t in range(NT + LAG):
                if t < NT:
                    emit_gather(t)
                if t >= LAG:
                    emit_combine(t - LAG)
                    if (t - LAG) % 4 == 3:
                        emit_matmuls((t - LAG) // 4)
            osb_ctx.__exit__(None, None, None)
            outp_ctx.__exit__(None, None, None)
            vtp_ctx.__exit__(None, None, None)
            pp_ctx.__exit__(None, None, None)
            vp_ctx.__exit__(None, None, None)
            gp_ctx.__exit__(None, None, None)
            fup_ctx.__exit__(None, None, None)
            trctx.__exit__(None, None, None)
            convtr.__exit__(None, None, None)
            setupctx.__exit__(None, None, None)
    _split_multi_waits(nc)
    return nc


def _split_multi_waits(nc, maxw=1):
    """The walrus build here rejects instructions with >1 semaphore wait;
    hoist excess waits onto standalone event-semaphore instructions."""
    n_fixed = 0
    for fn in nc.m.functions:
        for blk in fn.blocks:
            il = blk.instructions
            i = 0
            while i < len(il):
                inst = il[i]
                si = inst.sync_info
                if si is not None and len(si.on_wait) > maxw:
                    waits = list(si.on_wait)
                    keep = waits[:maxw - 1] if maxw > 1 else []
                    hoist = waits[len(keep):-1] if maxw > 1 else waits[:-1]
                    inst.sync_info = mybir.SyncInfo(
                        on_wait=keep + [waits[-1]], on_update=list(si.on_update)
                    )
                    for j, w in enumerate(hoist):
                        ev = mybir.InstEventSemaphore(
                            name=f"{inst.name}-hw{j}", ins=[], outs=[]
                        )
                        ev.engine = inst.engine
                        ev.sync_info = mybir.SyncInfo(on_wait=[w], on_update=[])
                        il.insert(i, ev)
                        i += 1
                    n_fixed += 1
                i += 1
    return n_fixed


# ---------------- host side ----------------

def _prep_inputs(input_x, w_off, b_off, w, b, gamma, beta, rmean, rvar):
    B = input_x.shape[0]
    x = np.asarray(input_x, np.float32)
    xbf = x.astype(ml_dtypes.bfloat16)
    xp = np.zeros((B, 256, HP, HP), ml_dtypes.bfloat16)
    xp[:, :, PAD:PAD + H, PAD:PAD + H] = xbf
    xpp = np.zeros((B, 256, HP + 1, HP + 1), ml_dtypes.bfloat16)
    xpp[:, :, :HP, :HP] = xp
    ys, xs = np.divmod(np.arange(NPIX), HP)
    xq = np.empty((B, NPIX, 4, 256), ml_dtypes.bfloat16)
    for j, (dy, dx) in enumerate(((0, 0), (1, 0), (0, 1), (1, 1))):
        xq[:, :, j, :] = xpp[:, :, ys + dy, xs + dx].transpose(0, 2, 1)
    xq = xq.reshape(B, NPIX, 1024)

    wofft = np.empty((128, 9, 2, 27), ml_dtypes.bfloat16)
    wf = np.asarray(w_off, np.float32)
    for ti in range(9):
        ty, tx = divmod(ti, 3)
        for ch in range(2):
            wofft[:, ti, ch, :] = wf[:, ch * 128:(ch + 1) * 128, ty, tx].T.astype(
                ml_dtypes.bfloat16)
    wofft = wofft.reshape(128, 9 * 2 * 27)

    scale_o = (np.asarray(gamma, np.float32)
               / np.sqrt(np.asarray(rvar, np.float32) + BN_EPS))
    wr = np.asarray(w, np.float32).reshape(256, 256, 9)
    wr = wr * scale_o[:, None, None]  # fold BN scale into conv weights
    w2 = np.empty((128, 18, 2, 128), ml_dtypes.bfloat16)
    for kk in range(9):
        for ch in range(2):
            j = 2 * kk + ch
            for oh in range(2):
                w2[:, j, oh, :] = wr[oh * 128:(oh + 1) * 128,
                                     ch * 128:(ch + 1) * 128, kk].T.astype(
                    ml_dtypes.bfloat16)
    w2 = w2.reshape(128, 18 * 2 * 128)

    ident = np.eye(128, dtype=np.float32)

    scale = (np.asarray(gamma, np.float32)
             / np.sqrt(np.asarray(rvar, np.float32) + BN_EPS))
    bias_tot = (np.asarray(b, np.float32) * scale
                + np.asarray(beta, np.float32)
                - np.asarray(rmean, np.float32) * scale)
    bnw = scale.reshape(2, 128).T.copy()  # unused on device now
    bnb = bias_tot.reshape(2, 128).T.copy()

    ky = (np.arange(K) // 3 - 1).astype(np.float32)
    kx = (np.arange(K) % 3 - 1).astype(np.float32)
    boff = np.asarray(b_off, np.float32)

    per_core = []
    for core in range(8):
        bidx_core, half = divmod(core, 2)
        h0 = half * 48
        # packed positions: p -> (y, x) = (p//96, p%96); padded coords
        # (h0+PAD+y, PAD+x)
        p = np.arange(128)[:, None] + 128 * np.arange(NT)[None, :]  # (128, NT)
        ypad = h0 + PAD + p // 96
        xpad = PAD + p % 96
        basey = (ypad[:, :, None] + ky[None, None, :]
                 + boff[0:18:2][None, None, :]).astype(np.float32)
        basex = (xpad[:, :, None] + kx[None, None, :]
                 + boff[1:18:2][None, None, :]).astype(np.float32)
        basem = np.broadcast_to(boff[18:27][None, None, :], basey.shape).astype(np.float32)
        slab = np.ascontiguousarray(
            xp[bidx_core, :, h0 + 4:h0 + 4 + SLAB_ROWS, :].reshape(256, SLAB)
            .reshape(2, 128, SLAB))
        per_core.append({
            "xq": np.ascontiguousarray(xq[bidx_core]),
            "xcm": slab,
            "wofft": wofft,
            "w2": w2,
            "ident": ident,
            "basey": np.ascontiguousarray(basey.reshape(128, NK)),
            "basex": np.ascontiguousarray(basex.reshape(128, NK)),
            "basem": np.ascontiguousarray(basem.reshape(128, NK)),
            "bnw": np.ascontiguousarray(bnw),
            "bnb": np.ascontiguousarray(bnb),
        })
    return per_core


_PROG_CACHE = {}


def _get_program():
    if "nc" not in _PROG_CACHE:
        _PROG_CACHE["nc"] = _build_program()
    return _PROG_CACHE["nc"]


def kernel(**inputs):
    return _run(inputs, trace=False)[0]


def _run(inputs, trace=False):
    per_core = _prep_inputs(**inputs)
    nc = _get_program()
    res = run_bass_kernel_spmd(nc, per_core, list(range(8)), trace=trace)
    out = np.empty((4, 256, 96, 96), np.float32)
    for core in range(8):
        bidx_core, half = divmod(core, 2)
        h0 = half * 48
        out[bidx_core, :, h0:h0 + 48, :] = (
            res.results[core]["out"].astype(np.float32).reshape(256, 48, 96))
    return out, res.exec_time_ns


# revision 4
# speedup vs baseline: 1.2706x; 1.0260x over previous
"""Modulated deformable conv (DCNv2) + eval-BN + ReLU on 8 TRN2 NeuronCores.

Sharding: 8 cores = (batch b in 0..3) x (image half h0 in {0, 48}).
Each core computes out[b, :, h0:h0+48, :] independently (no collectives).

Per-core pipeline (positions packed to 48x96 = 36 tiles of 128):
  - offset conv (3x3, 27ch) emitted as 6 independent units (conv matmuls ->
    PE pos-transposes -> DVE field math -> tap weights wq + gather indices
    idxu); all tile pools coexist so unit 0's gathers overlap later units
  - gather: per (tile, tap) one indirect SWDGE DMA fetches a 2KB "quad row"
    (4 bilinear corners x 256ch, bf16) per partition from a host-built table
  - tap combine per tile: 4 per-partition-scalar products (21 DVE
    tensor_scalar in fast mode + 15 ACT activation-scale) + 3 wide DVE adds
    -> val (pos-major, bf16)
  - val -> valt (ck, pos) via PE transposes (4 [128,128] chunks per j-chunk
    into one [128,512] PSUM tile, copied to SBUF on alternating DVE/ACT);
    keeps the DMA engines free for the gather stream
  - main conv: 36 accumulating matmuls per 4-tile group (N=512); BN scale
    folded into the conv weights host-side; bias+ReLU on ACT; bf16 out,
    one DMA per group.
"""

import numpy as np
import ml_dtypes

import concourse.bass as bass
import concourse.tile as tile
import concourse.mybir as mybir
from concourse.bass_utils import run_bass_kernel_spmd

bf16 = mybir.dt.bfloat16
f32 = mybir.dt.float32
i16 = mybir.dt.int16

K = 9
PAD = 6
H = 96
HP = H + 2 * PAD  # 108
NPIX = HP * HP  # 11664
NT = 36  # pos tiles of 128 over packed 48x96
NPOS = NT * 128  # 4608
NK = NT * K  # 324
SLAB_ROWS = 53
SLAB = SLAB_ROWS * HP
NCHUNK = 12  # offset-conv chunks of 4 rows (384 pos)
CHUNK = 384
BN_EPS = 1e-5

_AF = mybir.ActivationFunctionType
_ALU = mybir.AluOpType

CLOSE_SETUP = True
TEST_PLAIN_DMA = False
TR_ENGINE = None
OUT_ENGINE = None
GP_BUFS = 2
VAL_BUFS = 2


def _build_program():
    nc = bass.Bass()
    xq_e = nc.dram_tensor("xq", [NPIX, 1024], bf16, kind="ExternalInput")
    xcm_e = nc.dram_tensor("xcm", [2, 128, SLAB], bf16, kind="ExternalInput")
    wofft_e = nc.dram_tensor("wofft", [128, 9 * 2 * 27], bf16, kind="ExternalInput")
    w2_e = nc.dram_tensor("w2", [128, 18 * 2 * 128], bf16, kind="ExternalInput")
    ident_e = nc.dram_tensor("ident", [128, 128], f32, kind="ExternalInput")
    basey_e = nc.dram_tensor("basey", [128, NK], f32, kind="ExternalInput")
    basex_e = nc.dram_tensor("basex", [128, NK], f32, kind="ExternalInput")
    basem_e = nc.dram_tensor("basem", [128, NK], f32, kind="ExternalInput")
    bnw_e = nc.dram_tensor("bnw", [128, 2], f32, kind="ExternalInput")
    bnb_e = nc.dram_tensor("bnb", [128, 2], f32, kind="ExternalInput")
    out_e = nc.dram_tensor("out", [256, NPOS], bf16, kind="ExternalOutput")

    with tile.TileContext(nc) as tc:
        with (
            tc.tile_pool(name="const", bufs=1) as cp,
        ):
            setupctx = tc.tile_pool(name="setup", bufs=1)
            fp = setupctx.__enter__()
            # ---- load constants ----
            xcm = [fp.tile([128, SLAB], bf16, name=f"xcm{c}", tag=f"xcm{c}") for c in range(2)]
            _xsp = [0, 19 * HP, 37 * HP, SLAB]
            for c in range(2):
                for _a, _b in zip(_xsp[:-1], _xsp[1:]):
                    nc.sync.dma_start(xcm[c][:, _a:_b], xcm_e[c, :, _a:_b])
            wofft = fp.tile([128, 9 * 2 * 27], bf16)
            nc.sync.dma_start(wofft[:], wofft_e[:])
            w2 = cp.tile([128, 18 * 2 * 128], bf16)
            nc.sync.dma_start(w2[:], w2_e[:])
            ident = fp.tile([128, 128], f32)
            nc.sync.dma_start(ident[:], ident_e[:])
            basey = fp.tile([128, NK], f32)
            nc.sync.dma_start(basey[:], basey_e[:])
            basex = fp.tile([128, NK], f32)
            nc.sync.dma_start(basex[:], basex_e[:])
            basem = fp.tile([128, NK], f32)
            nc.sync.dma_start(basem[:], basem_e[:])
            bnw = cp.tile([128, 2], f32)
            nc.sync.dma_start(bnw[:], bnw_e[:])
            bnb = cp.tile([128, 2], f32)
            nc.sync.dma_start(bnb[:], bnb_e[:])

            # ---- setup: 6 units of 6 tiles, small rotating per-unit tiles so
            # the setup pool coexists with the main-loop pools (no closure
            # barrier -- gathers start right after unit 0's field math). ----
            convtr = tc.tile_pool(name="conv_ps", bufs=1, space="PSUM")
            convp = convtr.__enter__()
            trctx = tc.tile_pool(name="tr_ps", bufs=1, space="PSUM")
            trp = trctx.__enter__()
            fup_ctx = tc.tile_pool(name="funit", bufs=2)
            fup = fup_ctx.__enter__()
            idxu = cp.tile([128, NK], mybir.dt.uint32)
            wq = cp.tile([128, NK * 4], f32)
            identb = cp.tile([128, 128], bf16)
            nc.vector.tensor_copy(identb[:], ident[:])
            taps = [(dy, dx) for dy in (-1, 0, 1) for dx in (-1, 0, 1)]
            UT = 6            # tiles per unit
            UP = UT * 128     # positions per unit
            US = UT * K       # slots per unit

            def setup_half(h):
                off_u = fup.tile([32, UP], f32, name="off_u", tag="off_u")
                # offset conv for chunks [2h, 2h+2) -> off_u cols
                for ci in range(2):
                    r0 = (2 * h + ci) * 4
                    ps = convp.tile([32, CHUNK], f32, tag="convps")
                    n = 0
                    for ti, (dy, dx) in enumerate(taps):
                        for ch in range(2):
                            base = (2 + dy + r0) * HP + 6 + dx
                            rhs = xcm[ch][:, base:base + 4 * HP].rearrange(
                                "p (r w) -> p r w", w=HP)[:, :, :96]
                            nc.tensor.matmul(
                                ps[:27, :].rearrange("p (r w) -> p r w", w=96),
                                wofft[:, (ti * 2 + ch) * 27:(ti * 2 + ch) * 27 + 27],
                                rhs,
                                start=(n == 0),
                                stop=(n == 17),
                            )
                            n += 1
                    nc.vector.tensor_copy(
                        off_u[:27, ci * CHUNK:(ci + 1) * CHUNK], ps[:27, :])
                # transpose to pos-major for this unit's 6 tiles
                offpk = fup.tile([128, UT * 32], f32, name="offpk", tag="offpk")
                for tt in range(UT):
                    pst = trp.tile([128, 32], f32, tag="trps")
                    nc.tensor.transpose(
                        pst[:, :32], off_u[:32, tt * 128:(tt + 1) * 128],
                        ident[:32, :32])
                    nc.vector.tensor_copy(offpk[:, tt * 32:(tt + 1) * 32], pst[:])
                # field math on this unit's 54 slots
                o3 = offpk[:].rearrange("p (t c) -> p t c", c=32)
                dy_all = o3[:, :, 0:18:2]
                dx_all = o3[:, :, 1:18:2]
                ml_all = o3[:, :, 18:27]
                s = slice(US * h, US * h + US)
                pyt = fup.tile([128, US], f32, name="pyt", tag="pyt")
                pxt = fup.tile([128, US], f32, name="pxt", tag="pxt")
                fy = fup.tile([128, US], f32, name="fy", tag="fy")
                fx = fup.tile([128, US], f32, name="fx", tag="fx")
                y0 = fup.tile([128, US], f32, name="y0", tag="y0")
                x0 = fup.tile([128, US], f32, name="x0", tag="x0")
                msk = fup.tile([128, US], f32, name="msk", tag="msk")
                bb = fup.tile([128, US], f32, name="bb", tag="bb")
                aa = fup.tile([128, US], f32, name="aa", tag="aa")
                wx0 = fup.tile([128, US], f32, name="wx0", tag="wx0")
                idxf = fup.tile([128, US], f32, name="idxf", tag="idxf")
                yi = fup.tile([128, US], mybir.dt.int32, name="yi", tag="yi")
                xi = fup.tile([128, US], mybir.dt.int32, name="xi", tag="xi")
                gt = fup.tile([128, US], f32, name="gt", tag="gt")

                def v3(t128):
                    return t128[:].rearrange("p (t k) -> p t k", k=K)

                bs = basey[:, s].rearrange("p (t k) -> p t k", k=K)
                nc.vector.tensor_add(v3(pyt), dy_all, bs)
                bs = basex[:, s].rearrange("p (t k) -> p t k", k=K)
                nc.vector.tensor_add(v3(pxt), dx_all, bs)
                nc.vector.tensor_copy(yi[:], pyt[:])
                nc.vector.tensor_copy(y0[:], yi[:])
                nc.vector.tensor_tensor(gt[:], y0[:], pyt[:], op=_ALU.is_gt)
                nc.vector.tensor_sub(y0[:], y0[:], gt[:])
                nc.vector.tensor_sub(fy[:], pyt[:], y0[:])
                nc.vector.tensor_copy(xi[:], pxt[:])
                nc.vector.tensor_copy(x0[:], xi[:])
                nc.vector.tensor_tensor(gt[:], x0[:], pxt[:], op=_ALU.is_gt)
                nc.vector.tensor_sub(x0[:], x0[:], gt[:])
                nc.vector.tensor_sub(fx[:], pxt[:], x0[:])
                nc.vector.tensor_scalar(y0[:], y0[:], 0.0, float(HP - 2), op0=_ALU.max, op1=_ALU.min)
                nc.vector.tensor_scalar(x0[:], x0[:], 0.0, float(HP - 2), op0=_ALU.max, op1=_ALU.min)
                nc.vector.tensor_scalar(idxf[:], y0[:], float(HP), None, op0=_ALU.mult)
                nc.vector.tensor_add(idxf[:], idxf[:], x0[:])
                nc.vector.tensor_copy(idxu[:, s], idxf[:])
                bs = basem[:, s].rearrange("p (t k) -> p t k", k=K)
                nc.vector.tensor_add(v3(msk), ml_all, bs)
                nc.scalar.activation(msk[:], msk[:], _AF.Sigmoid)
                nc.vector.tensor_mul(bb[:], msk[:], fy[:])
                nc.vector.tensor_sub(aa[:], msk[:], bb[:])
                nc.vector.tensor_scalar(wx0[:], fx[:], -1.0, 1.0, op0=_ALU.mult, op1=_ALU.add)
                w3h = wq[:, 4 * US * h:4 * US * (h + 1)].rearrange(
                    "p (n j) -> p n j", j=4)
                nc.vector.tensor_mul(w3h[:, :, 0], aa[:], wx0[:])
                nc.vector.tensor_mul(w3h[:, :, 1], bb[:], wx0[:])
                nc.vector.tensor_mul(w3h[:, :, 2], aa[:], fx[:])
                nc.vector.tensor_mul(w3h[:, :, 3], bb[:], fx[:])

            gp_ctx = tc.tile_pool(name="gpool", bufs=GP_BUFS)
            gp = gp_ctx.__enter__()
            vp_ctx = tc.tile_pool(name="val", bufs=6)
            vp = vp_ctx.__enter__()
            pp_ctx = tc.tile_pool(name="prod", bufs=VAL_BUFS)
            pp = pp_ctx.__enter__()
            vtp_ctx = tc.tile_pool(name="valt", bufs=2)
            vtp = vtp_ctx.__enter__()
            outp_ctx = tc.tile_pool(name="out_ps", bufs=1, space="PSUM")
            outp = outp_ctx.__enter__()
            vtps_ctx = tc.tile_pool(name="vt_ps", bufs=4, space="PSUM")
            vtps_p = vtps_ctx.__enter__()
            osb_ctx = tc.tile_pool(name="osb", bufs=4)
            osb_p = osb_ctx.__enter__()
            for _h in range(6):
                setup_half(_h)


            # ---- main loop: software-pipelined with gathers LAG tiles ahead ----
            LAG = GP_BUFS - 1
            gbufs = {}
            valts = {}

            def emit_gather(t):
                g_t = gp.tile([128, 9 * 1024], bf16, tag="g")
                gbufs[t] = g_t
                for kk in range(K):
                    slot = t * K + kk
                    nc.gpsimd.indirect_dma_start(
                        out=g_t[:, kk * 1024:(kk + 1) * 1024],
                        out_offset=None,
                        in_=xq_e[:],
                        in_offset=bass.IndirectOffsetOnAxis(
                            ap=idxu[:, slot:slot + 1], axis=0
                        ),
                    )

            vals = {}

            def emit_combine(t):
                g_t = gbufs.pop(t)
                gg = t // 4
                u = t % 4
                val = vp.tile([128, 2304], bf16, name="val", tag="val")
                vals[u] = val
                p1 = pp.tile([128, 2304], bf16, tag="p1")
                p2 = pp.tile([128, 2304], bf16, tag="p2")
                p3 = pp.tile([128, 2304], bf16, tag="p3")
                for kk in range(K):
                    slot = t * K + kk
                    q = g_t[:, kk * 1024:(kk + 1) * 1024]
                    c = kk * 256
                    nc.vector.tensor_scalar(
                        val[:, c:c + 256], q[:, 0:256],
                        wq[:, slot * 4:slot * 4 + 1], None, op0=_ALU.mult)
                    if kk < 6:
                        nc.scalar.activation(
                            p1[:, c:c + 256], q[:, 256:512], _AF.Copy,
                            scale=wq[:, slot * 4 + 1:slot * 4 + 2])
                    else:
                        nc.vector.tensor_scalar(
                            p1[:, c:c + 256], q[:, 256:512],
                            wq[:, slot * 4 + 1:slot * 4 + 2], None, op0=_ALU.mult)
                    nc.scalar.activation(
                        p2[:, c:c + 256], q[:, 512:768], _AF.Copy,
                        scale=wq[:, slot * 4 + 2:slot * 4 + 3])
                    nc.vector.tensor_scalar(
                        p3[:, c:c + 256], q[:, 768:1024],
                        wq[:, slot * 4 + 3:slot * 4 + 4], None, op0=_ALU.mult)
                nc.vector.tensor_add(val[:], val[:], p1[:])
                nc.vector.tensor_add(p2[:], p2[:], p3[:])
                nc.vector.tensor_add(val[:], val[:], p2[:])
                if u == 3:
                    # PE-transpose the group's 4 val tiles into valt: per
                    # j-chunk J, 4 [128,128] transposes land in one [128,512]
                    # PSUM tile, then one copy casts it to bf16 SBUF.
                    valts[gg] = vtp.tile(
                        [128, 18 * 512], bf16, name="valt", tag="valt")
                    for J in range(18):
                        vt_ps = vtps_p.tile([128, 512], bf16, name="vt_ps", tag="vtps")
                        for u4 in range(4):
                            nc.tensor.transpose(
                                vt_ps[:, u4 * 128:(u4 + 1) * 128],
                                vals[u4][:, J * 128:(J + 1) * 128],
                                identb[:])
                        eng = nc.vector if J % 2 == 0 else nc.scalar
                        if J % 2 == 0:
                            nc.vector.tensor_copy(
                                valts[gg][:, J * 512:(J + 1) * 512], vt_ps[:])
                        else:
                            nc.scalar.activation(
                                valts[gg][:, J * 512:(J + 1) * 512], vt_ps[:],
                                _AF.Copy)
                    vals.clear()

            def emit_matmuls(g):
                tlo = g * 4
                valt = valts.pop(g)
                pso = [outp.tile([128, 512], f32, name=f"pso{oh}", tag=f"ops{oh}") for oh in range(2)]
                ob = osb_p.tile([128, 1024], bf16, tag="ob")
                for oh in range(2):
                    for j in range(18):
                        nc.tensor.matmul(
                            pso[oh][:],
                            w2[:, (j * 2 + oh) * 128:(j * 2 + oh) * 128 + 128],
                            valt[:, j * 512:(j + 1) * 512],
                            start=(j == 0),
                            stop=(j == 17),
                        )
                    nc.scalar.activation(
                        ob[:, oh * 512:(oh + 1) * 512], pso[oh][:], _AF.Relu,
                        bias=bnb[:, oh:oh + 1],
                    )
                nc.sync.dma_start(
                    out_e[:, tlo * 128:tlo * 128 + 512].rearrange(
                        "(oh p) n -> p oh n", oh=2),
                    ob[:].rearrange("p (oh n) -> p oh n", n=512),
                )

            for t in range(NT + LAG):
                if t < NT:
                    emit_gather(t)
                if t >= LAG:
                    emit_combine(t - LAG)
                    if (t - LAG) % 4 == 3:
                        emit_matmuls((t - LAG) // 4)
            osb_ctx.__exit__(None, None, None)
            vtps_ctx.__exit__(None, None, None)
            outp_ctx.__exit__(None, None, None)
            vtp_ctx.__exit__(None, None, None)
            pp_ctx.__exit__(None, None, None)
            vp_ctx.__exit__(None, None, None)
            gp_ctx.__exit__(None, None, None)
            fup_ctx.__exit__(None, None, None)
            trctx.__exit__(None, None, None)
            convtr.__exit__(None, None, None)
            setupctx.__exit__(None, None, None)
    _split_multi_waits(nc)
    return nc


def _split_multi_waits(nc, maxw=1):
    """The walrus build here rejects instructions with >1 semaphore wait;
    hoist excess waits onto standalone event-semaphore instructions."""
    n_fixed = 0
    for fn in nc.m.functions:
        for blk in fn.blocks:
            il = blk.instructions
            i = 0
            while i < len(il):
                inst = il[i]
                si = inst.sync_info
                if si is not None and len(si.on_wait) > maxw:
                    waits = list(si.on_wait)
                    keep = waits[:maxw - 1] if maxw > 1 else []
                    hoist = waits[len(keep):-1] if maxw > 1 else waits[:-1]
                    inst.sync_info = mybir.SyncInfo(
                        on_wait=keep + [waits[-1]], on_update=list(si.on_update)
                    )
                    for j, w in enumerate(hoist):
                        ev = mybir.InstEventSemaphore(
                            name=f"{inst.name}-hw{j}", ins=[], outs=[]
                        )
                        ev.engine = inst.engine
                        ev.sync_info = mybir.SyncInfo(on_wait=[w], on_update=[])
                        il.insert(i, ev)
                        i += 1
                    n_fixed += 1
                i += 1
    return n_fixed


# ---------------- host side ----------------

def _prep_inputs(input_x, w_off, b_off, w, b, gamma, beta, rmean, rvar):
    B = input_x.shape[0]
    x = np.asarray(input_x, np.float32)
    xbf = x.astype(ml_dtypes.bfloat16)
    xp = np.zeros((B, 256, HP, HP), ml_dtypes.bfloat16)
    xp[:, :, PAD:PAD + H, PAD:PAD + H] = xbf
    xpp = np.zeros((B, 256, HP + 1, HP + 1), ml_dtypes.bfloat16)
    xpp[:, :, :HP, :HP] = xp
    ys, xs = np.divmod(np.arange(NPIX), HP)
    xq = np.empty((B, NPIX, 4, 256), ml_dtypes.bfloat16)
    for j, (dy, dx) in enumerate(((0, 0), (1, 0), (0, 1), (1, 1))):
        xq[:, :, j, :] = xpp[:, :, ys + dy, xs + dx].transpose(0, 2, 1)
    xq = xq.reshape(B, NPIX, 1024)

    wofft = np.empty((128, 9, 2, 27), ml_dtypes.bfloat16)
    wf = np.asarray(w_off, np.float32)
    for ti in range(9):
        ty, tx = divmod(ti, 3)
        for ch in range(2):
            wofft[:, ti, ch, :] = wf[:, ch * 128:(ch + 1) * 128, ty, tx].T.astype(
                ml_dtypes.bfloat16)
    wofft = wofft.reshape(128, 9 * 2 * 27)

    scale_o = (np.asarray(gamma, np.float32)
               / np.sqrt(np.asarray(rvar, np.float32) + BN_EPS))
    wr = np.asarray(w, np.float32).reshape(256, 256, 9)
    wr = wr * scale_o[:, None, None]  # fold BN scale into conv weights
    w2 = np.empty((128, 18, 2, 128), ml_dtypes.bfloat16)
    for kk in range(9):
        for ch in range(2):
            j = 2 * kk + ch
            for oh in range(2):
                w2[:, j, oh, :] = wr[oh * 128:(oh + 1) * 128,
                                     ch * 128:(ch + 1) * 128, kk].T.astype(
                    ml_dtypes.bfloat16)
    w2 = w2.reshape(128, 18 * 2 * 128)

    ident = np.eye(128, dtype=np.float32)

    scale = (np.asarray(gamma, np.float32)
             / np.sqrt(np.asarray(rvar, np.float32) + BN_EPS))
    bias_tot = (np.asarray(b, np.float32) * scale
                + np.asarray(beta, np.float32)
                - np.asarray(rmean, np.float32) * scale)
    bnw = scale.reshape(2, 128).T.copy()  # unused on device now
    bnb = bias_tot.reshape(2, 128).T.copy()

    ky = (np.arange(K) // 3 - 1).astype(np.float32)
    kx = (np.arange(K) % 3 - 1).astype(np.float32)
    boff = np.asarray(b_off, np.float32)

    per_core = []
    for core in range(8):
        bidx_core, half = divmod(core, 2)
        h0 = half * 48
        # packed positions: p -> (y, x) = (p//96, p%96); padded coords
        # (h0+PAD+y, PAD+x)
        p = np.arange(128)[:, None] + 128 * np.arange(NT)[None, :]  # (128, NT)
        ypad = h0 + PAD + p // 96
        xpad = PAD + p % 96
        basey = (ypad[:, :, None] + ky[None, None, :]
                 + boff[0:18:2][None, None, :]).astype(np.float32)
        basex = (xpad[:, :, None] + kx[None, None, :]
                 + boff[1:18:2][None, None, :]).astype(np.float32)
        basem = np.broadcast_to(boff[18:27][None, None, :], basey.shape).astype(np.float32)
        slab = np.ascontiguousarray(
            xp[bidx_core, :, h0 + 4:h0 + 4 + SLAB_ROWS, :].reshape(256, SLAB)
            .reshape(2, 128, SLAB))
        per_core.append({
            "xq": np.ascontiguousarray(xq[bidx_core]),
            "xcm": slab,
            "wofft": wofft,
            "w2": w2,
            "ident": ident,
            "basey": np.ascontiguousarray(basey.reshape(128, NK)),
            "basex": np.ascontiguousarray(basex.reshape(128, NK)),
            "basem": np.ascontiguousarray(basem.reshape(128, NK)),
            "bnw": np.ascontiguousarray(bnw),
            "bnb": np.ascontiguousarray(bnb),
        })
    return per_core


_PROG_CACHE = {}


def _get_program():
    if "nc" not in _PROG_CACHE:
        _PROG_CACHE["nc"] = _build_program()
    return _PROG_CACHE["nc"]


def kernel(**inputs):
    return _run(inputs, trace=False)[0]


def _run(inputs, trace=False):
    per_core = _prep_inputs(**inputs)
    nc = _get_program()
    res = run_bass_kernel_spmd(nc, per_core, list(range(8)), trace=trace)
    out = np.empty((4, 256, 96, 96), np.float32)
    for core in range(8):
        bidx_core, half = divmod(core, 2)
        h0 = half * 48
        out[bidx_core, :, h0:h0 + 48, :] = (
            res.results[core]["out"].astype(np.float32).reshape(256, 48, 96))
    return out, res.exec_time_ns


# revision 6
# speedup vs baseline: 1.2920x; 1.0168x over previous
"""Modulated deformable conv (DCNv2) + eval-BN + ReLU on 8 TRN2 NeuronCores.

Sharding: 8 cores = (batch b in 0..3) x (image half h0 in {0, 48}).
Each core computes out[b, :, h0:h0+48, :] independently (no collectives).

v2 vs baseline:
  - positions packed to 48x96=4608 (36 tiles of 128), no pad-column waste
  - gather batched: one dma_gather per pos-tile fetches all 9 taps
    (1152 rows x 2KB quad) -> 36 SWDGE instructions instead of 369
  - wrapped int16 index layout built with 8 selection matmuls on PE
  - tap combine: 1 ACT scale-mul + 3 fused scalar_tensor_tensor on DVE
"""

import numpy as np
import ml_dtypes

import concourse.bass as bass
import concourse.tile as tile
import concourse.mybir as mybir
from concourse.bass_utils import run_bass_kernel_spmd

bf16 = mybir.dt.bfloat16
f32 = mybir.dt.float32
i16 = mybir.dt.int16

K = 9
PAD = 6
H = 96
HP = H + 2 * PAD  # 108
NPIX = HP * HP  # 11664
NT = 36  # pos tiles of 128 over packed 48x96
NPOS = NT * 128  # 4608
NK = NT * K  # 324
SLAB_ROWS = 53
SLAB = SLAB_ROWS * HP
NCHUNK = 12  # offset-conv chunks of 4 rows (384 pos)
CHUNK = 384
BN_EPS = 1e-5

_AF = mybir.ActivationFunctionType
_ALU = mybir.AluOpType

CLOSE_SETUP = True
TEST_PLAIN_DMA = False
TR_ENGINE = None
OUT_ENGINE = None
GP_BUFS = 2
VAL_BUFS = 2


def _build_program():
    nc = bass.Bass()
    xq_e = nc.dram_tensor("xq", [NPIX, 1024], bf16, kind="ExternalInput")
    xcm_e = nc.dram_tensor("xcm", [2, 128, SLAB], bf16, kind="ExternalInput")
    wofft_e = nc.dram_tensor("wofft", [128, 9 * 2 * 27], bf16, kind="ExternalInput")
    w2_e = nc.dram_tensor("w2", [128, 18 * 2 * 128], bf16, kind="ExternalInput")
    ident_e = nc.dram_tensor("ident", [128, 128], f32, kind="ExternalInput")
    basey_e = nc.dram_tensor("basey", [128, NK], f32, kind="ExternalInput")
    basex_e = nc.dram_tensor("basex", [128, NK], f32, kind="ExternalInput")
    basem_e = nc.dram_tensor("basem", [128, NK], f32, kind="ExternalInput")
    bnw_e = nc.dram_tensor("bnw", [128, 2], f32, kind="ExternalInput")
    bnb_e = nc.dram_tensor("bnb", [128, 2], f32, kind="ExternalInput")
    out_e = nc.dram_tensor("out", [256, NPOS], bf16, kind="ExternalOutput")

    with tile.TileContext(nc) as tc:
        with (
            tc.tile_pool(name="const", bufs=1) as cp,
        ):
            setupctx = tc.tile_pool(name="setup", bufs=1)
            fp = setupctx.__enter__()
            # ---- load constants ----
            xcm = [fp.tile([128, SLAB], bf16, name=f"xcm{c}", tag=f"xcm{c}") for c in range(2)]
            _xsp = [0, 19 * HP, 37 * HP, SLAB]
            for c in range(2):
                for _a, _b in zip(_xsp[:-1], _xsp[1:]):
                    nc.sync.dma_start(xcm[c][:, _a:_b], xcm_e[c, :, _a:_b])
            wofft = fp.tile([128, 9 * 2 * 27], bf16)
            nc.sync.dma_start(wofft[:], wofft_e[:])
            w2 = cp.tile([128, 18 * 2 * 128], bf16)
            nc.sync.dma_start(w2[:], w2_e[:])
            ident = fp.tile([128, 128], f32)
            nc.sync.dma_start(ident[:], ident_e[:])
            basey = fp.tile([128, NK], f32)
            nc.sync.dma_start(basey[:], basey_e[:])
            basex = fp.tile([128, NK], f32)
            nc.sync.dma_start(basex[:], basex_e[:])
            basem = fp.tile([128, NK], f32)
            nc.sync.dma_start(basem[:], basem_e[:])
            bnw = cp.tile([128, 2], f32)
            nc.sync.dma_start(bnw[:], bnw_e[:])
            bnb = cp.tile([128, 2], f32)
            nc.sync.dma_start(bnb[:], bnb_e[:])

            # ---- setup: 6 units of 6 tiles, small rotating per-unit tiles so
            # the setup pool coexists with the main-loop pools (no closure
            # barrier -- gathers start right after unit 0's field math). ----
            convtr = tc.tile_pool(name="conv_ps", bufs=1, space="PSUM")
            convp = convtr.__enter__()
            trctx = tc.tile_pool(name="tr_ps", bufs=1, space="PSUM")
            trp = trctx.__enter__()
            fup_ctx = tc.tile_pool(name="funit", bufs=2)
            fup = fup_ctx.__enter__()
            idxu = cp.tile([128, NK], mybir.dt.uint32)
            wq = cp.tile([128, NK * 4], f32)
            identb = cp.tile([128, 128], bf16)
            nc.vector.tensor_copy(identb[:], ident[:])
            taps = [(dy, dx) for dy in (-1, 0, 1) for dx in (-1, 0, 1)]
            UT = 6            # tiles per unit
            UP = UT * 128     # positions per unit
            US = UT * K       # slots per unit

            def setup_half(h):
                off_u = fup.tile([32, UP], f32, name="off_u", tag="off_u")
                # offset conv for chunks [2h, 2h+2) -> off_u cols
                for ci in range(2):
                    r0 = (2 * h + ci) * 4
                    ps = convp.tile([32, CHUNK], f32, tag="convps")
                    n = 0
                    for ti, (dy, dx) in enumerate(taps):
                        for ch in range(2):
                            base = (2 + dy + r0) * HP + 6 + dx
                            rhs = xcm[ch][:, base:base + 4 * HP].rearrange(
                                "p (r w) -> p r w", w=HP)[:, :, :96]
                            nc.tensor.matmul(
                                ps[:27, :].rearrange("p (r w) -> p r w", w=96),
                                wofft[:, (ti * 2 + ch) * 27:(ti * 2 + ch) * 27 + 27],
                                rhs,
                                start=(n == 0),
                                stop=(n == 17),
                            )
                            n += 1
                    nc.vector.tensor_copy(
                        off_u[:27, ci * CHUNK:(ci + 1) * CHUNK], ps[:27, :])
                # transpose to pos-major: all 6 tiles into one PSUM tile;
                # field math reads the PSUM directly (no offpk copies)
                pst6 = trp.tile([128, UT * 32], f32, name="pst6", tag="trps")
                for tt in range(UT):
                    nc.tensor.transpose(
                        pst6[:, tt * 32:(tt + 1) * 32],
                        off_u[:32, tt * 128:(tt + 1) * 128],
                        ident[:32, :32])
                # field math on this unit's 54 slots
                o3 = pst6[:].rearrange("p (t c) -> p t c", c=32)
                dy_all = o3[:, :, 0:18:2]
                dx_all = o3[:, :, 1:18:2]
                ml_all = o3[:, :, 18:27]
                s = slice(US * h, US * h + US)
                pyt = fup.tile([128, US], f32, name="pyt", tag="pyt")
                pxt = fup.tile([128, US], f32, name="pxt", tag="pxt")
                fy = fup.tile([128, US], f32, name="fy", tag="fy")
                fx = fup.tile([128, US], f32, name="fx", tag="fx")
                y0 = fup.tile([128, US], f32, name="y0", tag="y0")
                x0 = fup.tile([128, US], f32, name="x0", tag="x0")
                msk = fup.tile([128, US], f32, name="msk", tag="msk")
                bb = fup.tile([128, US], f32, name="bb", tag="bb")
                aa = fup.tile([128, US], f32, name="aa", tag="aa")
                wx0 = fup.tile([128, US], f32, name="wx0", tag="wx0")
                idxf = fup.tile([128, US], f32, name="idxf", tag="idxf")
                yi = fup.tile([128, US], mybir.dt.int32, name="yi", tag="yi")
                xi = fup.tile([128, US], mybir.dt.int32, name="xi", tag="xi")
                gt = fup.tile([128, US], f32, name="gt", tag="gt")

                def v3(t128):
                    return t128[:].rearrange("p (t k) -> p t k", k=K)

                bs = basey[:, s].rearrange("p (t k) -> p t k", k=K)
                nc.vector.tensor_add(v3(pyt), dy_all, bs)
                bs = basex[:, s].rearrange("p (t k) -> p t k", k=K)
                nc.vector.tensor_add(v3(pxt), dx_all, bs)
                nc.vector.tensor_copy(yi[:], pyt[:])
                nc.vector.tensor_copy(y0[:], yi[:])
                nc.vector.tensor_tensor(gt[:], y0[:], pyt[:], op=_ALU.is_gt)
                nc.vector.tensor_sub(y0[:], y0[:], gt[:])
                nc.vector.tensor_sub(fy[:], pyt[:], y0[:])
                nc.vector.tensor_copy(xi[:], pxt[:])
                nc.vector.tensor_copy(x0[:], xi[:])
                nc.vector.tensor_tensor(gt[:], x0[:], pxt[:], op=_ALU.is_gt)
                nc.vector.tensor_sub(x0[:], x0[:], gt[:])
                nc.vector.tensor_sub(fx[:], pxt[:], x0[:])
                nc.vector.tensor_scalar(y0[:], y0[:], 0.0, float(HP - 2), op0=_ALU.max, op1=_ALU.min)
                nc.vector.tensor_scalar(x0[:], x0[:], 0.0, float(HP - 2), op0=_ALU.max, op1=_ALU.min)
                nc.vector.tensor_scalar(idxf[:], y0[:], float(HP), None, op0=_ALU.mult)
                nc.vector.tensor_add(idxf[:], idxf[:], x0[:])
                nc.vector.tensor_copy(idxu[:, s], idxf[:])
                bs = basem[:, s].rearrange("p (t k) -> p t k", k=K)
                nc.vector.tensor_add(v3(msk), ml_all, bs)
                nc.scalar.activation(msk[:], msk[:], _AF.Sigmoid)
                nc.vector.tensor_mul(bb[:], msk[:], fy[:])
                nc.vector.tensor_sub(aa[:], msk[:], bb[:])
                nc.vector.tensor_scalar(wx0[:], fx[:], -1.0, 1.0, op0=_ALU.mult, op1=_ALU.add)
                w3h = wq[:, 4 * US * h:4 * US * (h + 1)].rearrange(
                    "p (n j) -> p n j", j=4)
                nc.vector.tensor_mul(w3h[:, :, 0], aa[:], wx0[:])
                nc.vector.tensor_mul(w3h[:, :, 1], bb[:], wx0[:])
                nc.vector.tensor_mul(w3h[:, :, 2], aa[:], fx[:])
                nc.vector.tensor_mul(w3h[:, :, 3], bb[:], fx[:])

            gp_ctx = tc.tile_pool(name="gpool", bufs=GP_BUFS)
            gp = gp_ctx.__enter__()
            vp_ctx = tc.tile_pool(name="val", bufs=6)
            vp = vp_ctx.__enter__()
            pp_ctx = tc.tile_pool(name="prod", bufs=VAL_BUFS)
            pp = pp_ctx.__enter__()
            vtp_ctx = tc.tile_pool(name="valt", bufs=2)
            vtp = vtp_ctx.__enter__()
            outp_ctx = tc.tile_pool(name="out_ps", bufs=1, space="PSUM")
            outp = outp_ctx.__enter__()
            vtps_ctx = tc.tile_pool(name="vt_ps", bufs=4, space="PSUM")
            vtps_p = vtps_ctx.__enter__()
            osb_ctx = tc.tile_pool(name="osb", bufs=4)
            osb_p = osb_ctx.__enter__()
            for _h in range(6):
                setup_half(_h)


            # ---- main loop: software-pipelined with gathers LAG tiles ahead ----
            LAG = GP_BUFS - 1
            gbufs = {}
            valts = {}

            def emit_gather(t):
                g_t = gp.tile([128, 9 * 1024], bf16, tag="g")
                gbufs[t] = g_t
                for kk in range(K):
                    slot = t * K + kk
                    nc.gpsimd.indirect_dma_start(
                        out=g_t[:, kk * 1024:(kk + 1) * 1024],
                        out_offset=None,
                        in_=xq_e[:],
                        in_offset=bass.IndirectOffsetOnAxis(
                            ap=idxu[:, slot:slot + 1], axis=0
                        ),
                    )

            vals = {}

            def emit_combine(t):
                g_t = gbufs.pop(t)
                gg = t // 4
                u = t % 4
                val = vp.tile([128, 2304], bf16, name="val", tag="val")
                vals[u] = val
                p1 = pp.tile([128, 2304], bf16, tag="p1")
                p2 = pp.tile([128, 2304], bf16, tag="p2")
                p3 = pp.tile([128, 2304], bf16, tag="p3")
                for kk in range(K):
                    slot = t * K + kk
                    q = g_t[:, kk * 1024:(kk + 1) * 1024]
                    c = kk * 256
                    nc.vector.tensor_scalar(
                        val[:, c:c + 256], q[:, 0:256],
                        wq[:, slot * 4:slot * 4 + 1], None, op0=_ALU.mult)
                    if kk < 6:
                        nc.scalar.activation(
                            p1[:, c:c + 256], q[:, 256:512], _AF.Copy,
                            scale=wq[:, slot * 4 + 1:slot * 4 + 2])
                    else:
                        nc.vector.tensor_scalar(
                            p1[:, c:c + 256], q[:, 256:512],
                            wq[:, slot * 4 + 1:slot * 4 + 2], None, op0=_ALU.mult)
                    nc.scalar.activation(
                        p2[:, c:c + 256], q[:, 512:768], _AF.Copy,
                        scale=wq[:, slot * 4 + 2:slot * 4 + 3])
                    nc.vector.tensor_scalar(
                        p3[:, c:c + 256], q[:, 768:1024],
                        wq[:, slot * 4 + 3:slot * 4 + 4], None, op0=_ALU.mult)
                nc.vector.tensor_add(val[:], val[:], p1[:])
                nc.vector.tensor_add(p2[:], p2[:], p3[:])
                nc.vector.tensor_add(val[:], val[:], p2[:])
                if u == 3:
                    # PE-transpose the group's 4 val tiles into valt: per
                    # j-chunk J, 4 [128,128] transposes land in one [128,512]
                    # PSUM tile, then one copy casts it to bf16 SBUF.
                    valts[gg] = vtp.tile(
                        [128, 18 * 512], bf16, name="valt", tag="valt")
                    for J2 in range(9):
                        vt_ps = vtps_p.tile([128, 1024], bf16, name="vt_ps", tag="vtps")
                        for jj in range(2):
                            J = 2 * J2 + jj
                            for u4 in range(4):
                                nc.tensor.transpose(
                                    vt_ps[:, jj * 512 + u4 * 128:jj * 512 + (u4 + 1) * 128],
                                    vals[u4][:, J * 128:(J + 1) * 128],
                                    identb[:])
                        if J2 % 2 == 0:
                            nc.vector.tensor_copy(
                                valts[gg][:, J2 * 1024:(J2 + 1) * 1024], vt_ps[:])
                        else:
                            nc.scalar.activation(
                                valts[gg][:, J2 * 1024:(J2 + 1) * 1024], vt_ps[:],
                                _AF.Copy)
                    vals.clear()

            def emit_matmuls(g):
                tlo = g * 4
                valt = valts.pop(g)
                pso = [outp.tile([128, 512], f32, name=f"pso{oh}", tag=f"ops{oh}") for oh in range(2)]
                ob = osb_p.tile([128, 1024], bf16, tag="ob")
                for oh in range(2):
                    for j in range(18):
                        nc.tensor.matmul(
                            pso[oh][:],
                            w2[:, (j * 2 + oh) * 128:(j * 2 + oh) * 128 + 128],
                            valt[:, j * 512:(j + 1) * 512],
                            start=(j == 0),
                            stop=(j == 17),
                        )
                    nc.scalar.activation(
                        ob[:, oh * 512:(oh + 1) * 512], pso[oh][:], _AF.Relu,
                        bias=bnb[:, oh:oh + 1],
                    )
                nc.sync.dma_start(
                    out_e[:, tlo * 128:tlo * 128 + 512].rearrange(
                        "(oh p) n -> p oh n", oh=2),
                    ob[:].rearrange("p (oh n) -> p oh n", n=512),
                )

            for t in range(NT + LAG):
                if t < NT:
                    emit_gather(t)
                if t >= LAG:
                    emit_combine(t - LAG)
                    if (t - LAG) % 4 == 3:
                        emit_matmuls((t - LAG) // 4)
            osb_ctx.__exit__(None, None, None)
            vtps_ctx.__exit__(None, None, None)
            outp_ctx.__exit__(None, None, None)
            vtp_ctx.__exit__(None, None, None)
            pp_ctx.__exit__(None, None, None)
            vp_ctx.__exit__(None, None, None)
            gp_ctx.__exit__(None, None, None)
            fup_ctx.__exit__(None, None, None)
            trctx.__exit__(None, None, None)
            convtr.__exit__(None, None, None)
            setupctx.__exit__(None, None, None)
    _split_multi_waits(nc)
    return nc


def _split_multi_waits(nc, maxw=1):
    """The walrus build here rejects instructions with >1 semaphore wait;
    hoist excess waits onto standalone event-semaphore instructions."""
    n_fixed = 0
    for fn in nc.m.functions:
        for blk in fn.blocks:
            il = blk.instructions
            i = 0
            while i < len(il):
                inst = il[i]
                si = inst.sync_info
                if si is not None and len(si.on_wait) > maxw:
                    waits = list(si.on_wait)
                    keep = waits[:maxw - 1] if maxw > 1 else []
                    hoist = waits[len(keep):-1] if maxw > 1 else waits[:-1]
                    inst.sync_info = mybir.SyncInfo(
                        on_wait=keep + [waits[-1]], on_update=list(si.on_update)
                    )
                    for j, w in enumerate(hoist):
                        ev = mybir.InstEventSemaphore(
                            name=f"{inst.name}-hw{j}", ins=[], outs=[]
                        )
                        ev.engine = inst.engine
                        ev.sync_info = mybir.SyncInfo(on_wait=[w], on_update=[])
                        il.insert(i, ev)
                        i += 1
                    n_fixed += 1
                i += 1
    return n_fixed


# ---------------- host side ----------------

def _prep_inputs(input_x, w_off, b_off, w, b, gamma, beta, rmean, rvar):
    B = input_x.shape[0]
    x = np.asarray(input_x, np.float32)
    xbf = x.astype(ml_dtypes.bfloat16)
    xp = np.zeros((B, 256, HP, HP), ml_dtypes.bfloat16)
    xp[:, :, PAD:PAD + H, PAD:PAD + H] = xbf
    xpp = np.zeros((B, 256, HP + 1, HP + 1), ml_dtypes.bfloat16)
    xpp[:, :, :HP, :HP] = xp
    ys, xs = np.divmod(np.arange(NPIX), HP)
    xq = np.empty((B, NPIX, 4, 256), ml_dtypes.bfloat16)
    for j, (dy, dx) in enumerate(((0, 0), (1, 0), (0, 1), (1, 1))):
        xq[:, :, j, :] = xpp[:, :, ys + dy, xs + dx].transpose(0, 2, 1)
    xq = xq.reshape(B, NPIX, 1024)

    wofft = np.empty((128, 9, 2, 27), ml_dtypes.bfloat16)
    wf = np.asarray(w_off, np.float32)
    for ti in range(9):
        ty, tx = divmod(ti, 3)
        for ch in range(2):
            wofft[:, ti, ch, :] = wf[:, ch * 128:(ch + 1) * 128, ty, tx].T.astype(
                ml_dtypes.bfloat16)
    wofft = wofft.reshape(128, 9 * 2 * 27)

    scale_o = (np.asarray(gamma, np.float32)
               / np.sqrt(np.asarray(rvar, np.float32) + BN_EPS))
    wr = np.asarray(w, np.float32).reshape(256, 256, 9)
    wr = wr * scale_o[:, None, None]  # fold BN scale into conv weights
    w2 = np.empty((128, 18, 2, 128), ml_dtypes.bfloat16)
    for kk in range(9):
        for ch in range(2):
            j = 2 * kk + ch
            for oh in range(2):
                w2[:, j, oh, :] = wr[oh * 128:(oh + 1) * 128,
                                     ch * 128:(ch + 1) * 128, kk].T.astype(
                    ml_dtypes.bfloat16)
    w2 = w2.reshape(128, 18 * 2 * 128)

    ident = np.eye(128, dtype=np.float32)

    scale = (np.asarray(gamma, np.float32)
             / np.sqrt(np.asarray(rvar, np.float32) + BN_EPS))
    bias_tot = (np.asarray(b, np.float32) * scale
                + np.asarray(beta, np.float32)
                - np.asarray(rmean, np.float32) * scale)
    bnw = scale.reshape(2, 128).T.copy()  # unused on device now
    bnb = bias_tot.reshape(2, 128).T.copy()

    ky = (np.arange(K) // 3 - 1).astype(np.float32)
    kx = (np.arange(K) % 3 - 1).astype(np.float32)
    boff = np.asarray(b_off, np.float32)

    per_core = []
    for core in range(8):
        bidx_core, half = divmod(core, 2)
        h0 = half * 48
        # packed positions: p -> (y, x) = (p//96, p%96); padded coords
        # (h0+PAD+y, PAD+x)
        p = np.arange(128)[:, None] + 128 * np.arange(NT)[None, :]  # (128, NT)
        ypad = h0 + PAD + p // 96
        xpad = PAD + p % 96
        basey = (ypad[:, :, None] + ky[None, None, :]
                 + boff[0:18:2][None, None, :]).astype(np.float32)
        basex = (xpad[:, :, None] + kx[None, None, :]
                 + boff[1:18:2][None, None, :]).astype(np.float32)
        basem = np.broadcast_to(boff[18:27][None, None, :], basey.shape).astype(np.float32)
        slab = np.ascontiguousarray(
            xp[bidx_core, :, h0 + 4:h0 + 4 + SLAB_ROWS, :].reshape(256, SLAB)
            .reshape(2, 128, SLAB))
        per_core.append({
            "xq": np.ascontiguousarray(xq[bidx_core]),
            "xcm": slab,
            "wofft": wofft,
            "w2": w2,
            "ident": ident,
            "basey": np.ascontiguousarray(basey.reshape(128, NK)),
            "basex": np.ascontiguousarray(basex.reshape(128, NK)),
            "basem": np.ascontiguousarray(basem.reshape(128, NK)),
            "bnw": np.ascontiguousarray(bnw),
            "bnb": np.ascontiguousarray(bnb),
        })
    return per_core


_PROG_CACHE = {}


def _get_program():
    if "nc" not in _PROG_CACHE:
        _PROG_CACHE["nc"] = _build_program()
    return _PROG_CACHE["nc"]


def kernel(**inputs):
    return _run(inputs, trace=False)[0]


def _run(inputs, trace=False):
    per_core = _prep_inputs(**inputs)
    nc = _get_program()
    res = run_bass_kernel_spmd(nc, per_core, list(range(8)), trace=trace)
    out = np.empty((4, 256, 96, 96), np.float32)
    for core in range(8):
        bidx_core, half = divmod(core, 2)
        h0 = half * 48
        out[bidx_core, :, h0:h0 + 48, :] = (
            res.results[core]["out"].astype(np.float32).reshape(256, 48, 96))
    return out, res.exec_time_ns
